# revision 1
# baseline (speedup 1.0000x reference)
"""DCRNN kernel for 8 Trainium2 NeuronCores (Bass/Tile).

Graph/data-parallel sharding: nodes permuted so core c owns batch-lanes
[c*125,(c+1)*125) of every graph; edges partitioned by dst shard and bucketed
by (dst-group of 128, src-block of 25000) with cross-core-uniform chunk counts
so one SPMD program serves all 8 cores. Aggregation = dma_gather (<=1024 idxs
per call) + one-hot matmul scatter in PSUM; the one-hot is built per chunk via
a single fused tensor_scalar (is_equal with the slot id, times recip[dst]), so
the deg-normalization costs nothing downstream. conv2 scatters in transposed
orientation (lhsT = gathered rows) giving aggT [H,128] directly — no PE
transpose. conv1's tiny 4-wide transposed agg is AllGathered; every core then
recomputes full h1 and writes a local bf16 h1 table for conv2's gather, with
the ReLU+cast split across Act and DVE in 512-wide batches. LSTM runs in bf16
(weights/activations; cell state fp32) with gates i,f,o batched into one
sigmoid per step-layer and biases folded in via a rank-3 matmul; it is emitted
interleaved with conv2 so the recurrence pipelines under the gather stream.
Global mean pool via free-dim reduce + AllReduce.
"""
import numpy as np
import ml_dtypes

BF16 = ml_dtypes.bfloat16

N = 100000
NPG = 1000
B_GRAPHS = 100
H = 128
CIN = 3
OUT = 2
NCORES = 8
SH = 12500          # nodes per core
NB = 4              # src blocks (int16 gather index limit)
BLK = 25000         # nodes per src block
NG = 98             # dst groups of 128 per core (last group = 84 nodes)
SHPAD = NG * 128    # 12544
GS = 3              # dst groups per super-group
T = 100
BL = 125            # batch lanes per core
GMAX = 1024         # max idxs per dma_gather on this runtime

_BUILT = {}
_NO_LSTM = False


# --------------------------------------------------------------------------
# host preprocessing
# --------------------------------------------------------------------------
def _perm():
    n = np.arange(N)
    c = (n % NPG) // BL
    return c * SH + (n // NPG) * BL + (n % NPG) % BL


def _host_prep(inputs):
    x = np.asarray(inputs["x"], np.float32)
    ei = np.asarray(inputs["edge_index"])
    src, dst = ei[0].astype(np.int64), ei[1].astype(np.int64)
    p = _perm()
    srcp = p[src]
    dstp = p[dst]

    deg = np.bincount(dstp, minlength=N).astype(np.float32)
    recip = (1.0 / np.maximum(deg, 1.0)).astype(np.float32)

    owner = dstp // SH
    K = np.zeros((NG, NB), np.int64)
    per_core = []
    for c in range(NCORES):
        m = owner == c
        L = dstp[m] - c * SH
        g = L // 128
        slot = (L % 128).astype(np.float32)
        rc = recip[dstp[m]]
        b = srcp[m] // BLK
        s16 = (srcp[m] % BLK).astype(np.int16)
        key = (g * NB + b).astype(np.int64)
        order = np.argsort(key, kind="stable")
        cnt = np.bincount(key, minlength=NG * NB)
        per_core.append((s16[order], slot[order], rc[order], key[order], cnt))
        K = np.maximum(K, ((cnt + 127) // 128).reshape(NG, NB))
    K = np.maximum(K, 1)

    # chunk layout: for sup: for b: for g in sup: K[g,b] chunks
    sups = [range(i, min(i + GS, NG)) for i in range(0, NG, GS)]
    chunk_base = np.zeros((NG, NB), np.int64)
    gmeta = []
    nch = 0
    for sup in sups:
        sup_base = nch
        bruns = []
        for b in range(NB):
            run_base = nch
            for g in sup:
                chunk_base[g, b] = nch
                nch += K[g, b]
            bruns.append((b, run_base * 128, (nch - run_base) * 128))
        gmeta.append((sup_base, nch - sup_base, bruns))
    NCH = nch
    NSL = NCH * 128

    percore = []
    base_of_key = chunk_base.reshape(-1) * 128
    for c in range(NCORES):
        s_sorted, slot_sorted, rc_sorted, key_sorted, cnt = per_core[c]
        run_start = np.concatenate([[0], np.cumsum(cnt)[:-1]])
        rank_within = np.arange(len(s_sorted)) - run_start[key_sorted]
        pos = base_of_key[key_sorted] + rank_within
        idx_flat = np.zeros(NSL, np.int16)
        dm_flat = np.full(NSL, -1.0, np.float32)
        rc_flat = np.zeros(NSL, np.float32)
        idx_flat[pos] = s_sorted
        dm_flat[pos] = slot_sorted
        rc_flat[pos] = rc_sorted
        w = idx_flat.reshape(NSL // 16, 16).T
        percore.append({
            "idx16": np.ascontiguousarray(np.tile(w, (8, 1)).astype(np.int16)),
            "dmv": np.ascontiguousarray(
                dm_flat.reshape(NCH, 128).T.astype(np.float32)),
            "recb": np.ascontiguousarray(
                rc_flat.reshape(NCH, 128).T.astype(np.float32)),
        })

    # tables / weights in perm order
    inv = np.empty(N, np.int64)
    inv[p] = np.arange(N)
    xp = np.zeros((N, H), np.float32)
    xp[:, :CIN] = x[inv]
    xp[:, CIN] = 1.0
    x4T = xp[:, :4].T.copy()
    for c in range(NCORES):
        xl = np.zeros((4, SHPAD), np.float32)
        xl[:, :SH] = x4T[:, c * SH:(c + 1) * SH]
        percore[c]["x4loc"] = np.ascontiguousarray(xl.astype(BF16))

    Wcomb = np.zeros((8, H), np.float32)
    Wcomb[0:3] = np.asarray(inputs["W_self0"], np.float32)
    Wcomb[3] = np.asarray(inputs["b0"], np.float32)
    Wcomb[4:7] = np.asarray(inputs["W_nbr0"], np.float32)

    shared = {
        "iotab": np.ascontiguousarray(
            np.broadcast_to(np.arange(128, dtype=np.float32), (128, 128))
            .astype(BF16)),
        "xtab": xp.astype(BF16),
        "x4T": np.ascontiguousarray(x4T.astype(BF16)),
        "wcomb": Wcomb.astype(BF16),
        "ws1": np.asarray(inputs["W_self1"], np.float32).astype(BF16),
        "wn1": np.asarray(inputs["W_nbr1"], np.float32).astype(BF16),
        "b1c": np.ascontiguousarray(
            np.asarray(inputs["b1"], np.float32).reshape(H, 1)),
        "wo": (np.asarray(inputs["W_out"], np.float32) / NPG)
            .astype(np.float32),
        "bo": np.ascontiguousarray(
            np.asarray(inputs["b_out"], np.float32).reshape(OUT, 1)),
    }
    # LSTM weights, gate column order [i, f, o, g] (torch layout i,f,g,o)
    QORD = (0, 1, 3, 2)
    gsel = np.zeros((3, 3 * BL), np.float32)
    for q in range(3):
        gsel[q, q * BL:(q + 1) * BL] = 1.0
    shared["gsel"] = gsel.astype(BF16)
    for l in range(2):
        wi = np.asarray(inputs[f"Wih{l}"], np.float32)
        wh = np.asarray(inputs[f"Whh{l}"], np.float32)
        bs = (np.asarray(inputs[f"bih{l}"], np.float32)
              + np.asarray(inputs[f"bhh{l}"], np.float32))
        shared[f"wi{l}"] = np.ascontiguousarray(np.concatenate(
            [wi[qt * H:(qt + 1) * H].T for qt in QORD], axis=1)).astype(BF16)
        shared[f"wh{l}"] = np.ascontiguousarray(np.concatenate(
            [wh[qt * H:(qt + 1) * H].T for qt in QORD], axis=1)).astype(BF16)
        bs4 = bs.reshape(4, H)[list(QORD)]          # [4, H] in i,f,o,g order
        shared[f"bs3{l}"] = np.ascontiguousarray(bs4[0:3]).astype(BF16)
        shared[f"bg{l}"] = np.ascontiguousarray(
            bs4[3].reshape(H, 1)).astype(np.float32)

    meta = tuple(K.reshape(-1).tolist())
    return shared, percore, meta, K, gmeta, NCH, chunk_base


# --------------------------------------------------------------------------
# device program
# --------------------------------------------------------------------------
def _build_nc(K, gmeta, NCH, chunk_base, stop_after=None):
    import concourse.bacc as bacc
    import concourse.mybir as mybir
    from concourse.tile import TileContext

    f32 = mybir.dt.float32
    bf = mybir.dt.bfloat16
    i16 = mybir.dt.int16
    AF = mybir.ActivationFunctionType
    ALU = mybir.AluOpType
    NSL = NCH * 128
    Kf = K.reshape(NG, NB)
    sups = [range(i, min(i + GS, NG)) for i in range(0, NG, GS)]

    nc = bacc.Bacc(None, target_bir_lowering=False)

    d_xtab = nc.dram_tensor("xtab", [N, H], bf, kind="ExternalInput")
    d_x4T = nc.dram_tensor("x4T", [4, N], bf, kind="ExternalInput")
    d_wcomb = nc.dram_tensor("wcomb", [8, H], bf, kind="ExternalInput")
    d_ws1 = nc.dram_tensor("ws1", [H, H], bf, kind="ExternalInput")
    d_wn1 = nc.dram_tensor("wn1", [H, H], bf, kind="ExternalInput")
    d_b1c = nc.dram_tensor("b1c", [H, 1], f32, kind="ExternalInput")
    d_wo = nc.dram_tensor("wo", [H, OUT], f32, kind="ExternalInput")
    d_bo = nc.dram_tensor("bo", [OUT, 1], f32, kind="ExternalInput")
    d_gsel = nc.dram_tensor("gsel", [3, 3 * BL], bf, kind="ExternalInput")
    d_wi = [nc.dram_tensor(f"wi{l}", [H, 4 * H], bf, kind="ExternalInput")
            for l in range(2)]
    d_wh = [nc.dram_tensor(f"wh{l}", [H, 4 * H], bf, kind="ExternalInput")
            for l in range(2)]
    d_bs3 = [nc.dram_tensor(f"bs3{l}", [3, H], bf, kind="ExternalInput")
             for l in range(2)]
    d_bg = [nc.dram_tensor(f"bg{l}", [H, 1], f32, kind="ExternalInput")
            for l in range(2)]
    d_idx = nc.dram_tensor("idx16", [128, NSL // 16], i16,
                           kind="ExternalInput")
    d_dmv = nc.dram_tensor("dmv", [128, NCH], f32, kind="ExternalInput")
    d_recb = nc.dram_tensor("recb", [128, NCH], f32, kind="ExternalInput")
    d_iotab = nc.dram_tensor("iotab", [128, 128], bf, kind="ExternalInput")
    d_x4loc = nc.dram_tensor("x4loc", [4, SHPAD], bf, kind="ExternalInput")
    d_out = nc.dram_tensor("out", [B_GRAPHS, OUT], f32, kind="ExternalOutput")

    with TileContext(nc) as tc:
        with (
            tc.tile_pool(name="dram", bufs=1, space="DRAM") as dramp,
            tc.tile_pool(name="persist", bufs=1) as pers,
        ):
            h1tab = dramp.tile([N, H], bf)
            cc_in = dramp.tile([4, SHPAD], bf)
            cc_out = dramp.tile([4 * NCORES, SHPAD], bf, addr_space="Shared")
            ccr_in = dramp.tile([OUT, B_GRAPHS], f32)
            ccr_out = dramp.tile([OUT, B_GRAPHS], f32, addr_space="Shared")

            dmvt = pers.tile([128, NCH], f32)
            recbt = pers.tile([128, NCH], f32)
            h1Tl = pers.tile([H, SHPAD], bf)
            h2T = pers.tile([H, SHPAD], bf)
            x4lt = pers.tile([4, SHPAD], bf)
            aggnT = pers.tile([4, SHPAD], bf)
            w_nb0 = pers.tile([4, H], bf)
            w_comb = pers.tile([8, H], bf)
            w_s1 = pers.tile([H, H], bf)
            w_n1 = pers.tile([H, H], bf)
            b1c = pers.tile([H, 1], f32)
            gsel = pers.tile([3, 3 * BL], bf)
            w_i = [pers.tile([H, 4 * H], bf, name=f"w_i{l}") for l in range(2)]
            w_h = [pers.tile([H, 4 * H], bf, name=f"w_h{l}") for l in range(2)]
            bs3 = [pers.tile([3, H], bf, name=f"bs3{l}") for l in range(2)]
            bg = [pers.tile([H, 1], f32, name=f"bg{l}") for l in range(2)]
            w_o = pers.tile([H, OUT], f32)
            b_o = pers.tile([OUT, 1], f32)
            iotab = pers.tile([128, 128], bf)
            identf = pers.tile([128, OUT], f32)
            pooledT = pers.tile([H, B_GRAPHS], f32)

            nc.sync.dma_start(out=dmvt[:], in_=d_dmv[:])
            nc.sync.dma_start(out=recbt[:], in_=d_recb[:])
            nc.sync.dma_start(out=x4lt[:], in_=d_x4loc[:])
            nc.sync.dma_start(out=w_nb0[:], in_=d_wcomb[4:8, :])
            nc.sync.dma_start(out=w_comb[:], in_=d_wcomb[:])
            nc.sync.dma_start(out=w_s1[:], in_=d_ws1[:])
            nc.sync.dma_start(out=w_n1[:], in_=d_wn1[:])
            nc.sync.dma_start(out=b1c[:], in_=d_b1c[:])
            nc.sync.dma_start(out=gsel[:], in_=d_gsel[:])
            for l in range(2):
                nc.sync.dma_start(out=w_i[l][:], in_=d_wi[l][:])
                nc.sync.dma_start(out=w_h[l][:], in_=d_wh[l][:])
                nc.sync.dma_start(out=bs3[l][:], in_=d_bs3[l][:])
                nc.sync.dma_start(out=bg[l][:], in_=d_bg[l][:])
            nc.sync.dma_start(out=w_o[:], in_=d_wo[:])
            nc.sync.dma_start(out=b_o[:], in_=d_bo[:])
            nc.sync.dma_start(out=iotab[:], in_=d_iotab[:])
            from concourse.masks import make_identity
            make_identity(nc, identf[0:OUT, 0:OUT])

            def conv_phase(gpool, ohpool, ipool, table, emit_group,
                           emit_sup, lhs_w):
                """Shared phase-1/phase-4 edge-aggregation loop."""
                for (sup_base, nch_sup, bruns), sup in zip(gmeta, sups):
                    oh_t = ohpool.tile([128, nch_sup * 128], bf, tag="oh")
                    for k in range(nch_sup):
                        c = sup_base + k
                        nc.vector.tensor_scalar(
                            out=oh_t[:, k * 128:(k + 1) * 128],
                            in0=iotab[:],
                            scalar1=dmvt[:, c:c + 1],
                            scalar2=recbt[:, c:c + 1],
                            op0=ALU.is_equal, op1=ALU.mult)
                    g_tiles = {}
                    for b, slot_base, n_idx in bruns:
                        it = ipool.tile([128, n_idx // 16], i16, tag=f"i{b}")
                        nc.sync.dma_start(
                            out=it[:],
                            in_=d_idx[:, slot_base // 16:
                                      (slot_base + n_idx) // 16])
                        gt = gpool.tile([128, n_idx], bf, tag=f"g{b}")
                        for o in range(0, n_idx, GMAX):
                            nn_ = min(GMAX, n_idx - o)
                            nc.gpsimd.dma_gather(
                                out_ap=gt[:, o:o + nn_]
                                    .rearrange("p (k h) -> p k h", h=H),
                                in_ap=table[b * BLK:(b + 1) * BLK, :],
                                idxs_ap=it[:, o // 16:(o + nn_) // 16],
                                num_idxs=nn_,
                                num_idxs_reg=nn_,
                                elem_size=H,
                            )
                        g_tiles[b] = (gt, slot_base)
                    for g in sup:
                        mms = []
                        for b in range(NB):
                            gt, slot_base = g_tiles[b]
                            for kk in range(Kf[g, b]):
                                chunk = int(chunk_base[g, b]) + kk
                                oh_ap = oh_t[:, (chunk - sup_base) * 128:
                                             (chunk - sup_base + 1) * 128]
                                off = chunk * 128 - slot_base
                                g_ap = gt[:, off:off + lhs_w]
                                first = (b == 0 and kk == 0)
                                last = (b == NB - 1 and kk == Kf[g, b] - 1)
                                mms.append((oh_ap, g_ap, first, last))
                        emit_group(g, mms)
                    emit_sup(sup_base, sup)

            # ---------------- Phase 1: conv1 aggregation -----------------
            with (
                tc.tile_pool(name="p1g", bufs=2) as gpool,
                tc.tile_pool(name="p1oh", bufs=2) as ohpool,
                tc.tile_pool(name="p1i", bufs=2) as ipool1,
                tc.tile_pool(name="p1ps", bufs=2, space="PSUM") as pspool,
            ):
                cur = {}
                junk = pers.tile([4, 1], f32)
                nc.vector.tensor_tensor(out=junk[:], in0=iotab[0:4, 0:1],
                                        in1=dmvt[0:4, 0:1], op=ALU.add)

                def emit_group1(g, mms):
                    gis = g % GS
                    if gis == 0:
                        cur["ps"] = pspool.tile([4, GS * 128], f32, space="PSUM",
                                                tag="aggps", name="aggps")
                    ps = cur["ps"]
                    for oh_ap, g_ap, first, last in mms:
                        nc.tensor.matmul(
                            out=ps[:, gis * 128:(gis + 1) * 128],
                            lhsT=g_ap, rhs=oh_ap,
                            start=first, stop=last)

                def emit_sup1(sup_base, sup):
                    g0 = sup[0]
                    w = (len(sup)) * 128
                    nc.vector.tensor_copy(
                        out=aggnT[:, g0 * 128:g0 * 128 + w],
                        in_=cur["ps"][:, :w])

                conv_phase(gpool, ohpool, ipool1, d_xtab,
                           emit_group1, emit_sup1, 4)

            nc.sync.dma_start(out=cc_in[:], in_=aggnT[:])
            nc.gpsimd.collective_compute(
                "AllGather", mybir.AluOpType.bypass,
                replica_groups=[list(range(NCORES))],
                ins=[cc_in.opt()], outs=[cc_out.opt()],
            )

            if stop_after is None or stop_after >= 3:
                # -------- Phase 3: own h1Tl + full h1tab recompute -----------
                with (
                    tc.tile_pool(name="p3xa", bufs=2) as xapool,
                    tc.tile_pool(name="p3r", bufs=2) as rpool,
                    tc.tile_pool(name="p3ps", bufs=2, space="PSUM") as pspool3,
                    tc.tile_pool(name="p3ps2", bufs=2, space="PSUM") as pspool3b,
                ):
                    # own shard, feature-major, bf16 (conv2 self term)
                    for c0 in range(0, SHPAD, 512):
                        w = min(512, SHPAD - c0)
                        ps = pspool3.tile([H, 512], f32, space="PSUM", tag="own")
                        nc.tensor.matmul(out=ps[:, :w], lhsT=w_comb[0:4, :],
                                         rhs=x4lt[:, c0:c0 + w],
                                         start=True, stop=False)
                        nc.tensor.matmul(out=ps[:, :w], lhsT=w_nb0[:],
                                         rhs=aggnT[:, c0:c0 + w],
                                         start=False, stop=True)
                        nc.scalar.activation(h1Tl[:, c0:c0 + w], ps[:, :w],
                                             AF.Relu)

                    # all ranks, node-major rows -> h1tab: per-rank bulk loads,
                    # one bulk write per rank (+tail), Act/DVE-alternating relu
                    for r in range(NCORES):
                        xa = xapool.tile([8, SH], bf, tag="xa")
                        nc.sync.dma_start(out=xa[0:4, :],
                                          in_=d_x4T[:, r * SH:(r + 1) * SH])
                        nc.sync.dma_start(out=xa[4:8, :],
                                          in_=cc_out[4 * r:4 * r + 4, 0:SH])
                        for half in range(2):
                            hq = half * 48
                            rowb = rpool.tile([128, 48 * H], bf, tag="rowb")
                            for q0 in range(hq, hq + 48, 4):
                                ps = pspool3b.tile([128, 512], f32, space="PSUM",
                                                   tag="rows")
                                for gi in range(4):
                                    nc.tensor.matmul(
                                        out=ps[:, gi * H:(gi + 1) * H],
                                        lhsT=xa[:, (q0 + gi) * 128:
                                                (q0 + gi) * 128 + 128],
                                        rhs=w_comb[:], start=True, stop=True)
                                if (q0 // 4) % 2 == 0:
                                    nc.scalar.activation(
                                        rowb[:, (q0 - hq) * H:(q0 - hq + 4) * H],
                                        ps[:], AF.Relu)
                                else:
                                    nc.vector.tensor_scalar(
                                        out=rowb[:, (q0 - hq) * H:
                                                 (q0 - hq + 4) * H],
                                        in0=ps[:],
                                        scalar1=0.0, scalar2=None, op0=ALU.max)
                            nc.sync.dma_start(
                                out=h1tab[r * SH + hq * 128:
                                          r * SH + (hq + 48) * 128, :]
                                    .rearrange("(j p) h -> p j h", p=128),
                                in_=rowb[:, :].rearrange("p (j h) -> p j h", h=H))
                        # tail: groups 96 (full) and 97 (84 rows)
                        pst = pspool3b.tile([128, 512], f32, space="PSUM",
                                            tag="rows")
                        nc.tensor.matmul(out=pst[:, 0:H],
                                         lhsT=xa[:, 96 * 128:97 * 128],
                                         rhs=w_comb[:], start=True, stop=True)
                        nc.tensor.matmul(out=pst[0:84, H:2 * H],
                                         lhsT=xa[:, 97 * 128:SH],
                                         rhs=w_comb[:], start=True, stop=True)
                        rowt = rpool.tile([128, 2 * H], bf, tag="rowt")
                        nc.scalar.activation(rowt[:, 0:H], pst[:, 0:H], AF.Relu)
                        nc.scalar.activation(rowt[0:84, H:2 * H],
                                             pst[0:84, H:2 * H], AF.Relu)
                        nc.sync.dma_start(
                            out=h1tab[r * SH + 96 * 128:r * SH + 97 * 128, :],
                            in_=rowt[:, 0:H])
                        nc.sync.dma_start(
                            out=h1tab[r * SH + 97 * 128:(r + 1) * SH, :],
                            in_=rowt[0:84, H:2 * H])

            if stop_after is None or stop_after >= 4:
                # ------- Phase 4: conv2 + interleaved LSTM -------------------
                with (
                    tc.tile_pool(name="p4g", bufs=2) as gpool4,
                    tc.tile_pool(name="p4oh", bufs=2) as ohpool4,
                    tc.tile_pool(name="p4i", bufs=1) as ipool4,
                    tc.tile_pool(name="p4ps", bufs=2, space="PSUM") as pspool4a,
                    tc.tile_pool(name="p4ps2", bufs=2, space="PSUM") as pspool4b,
                    tc.tile_pool(name="p4t", bufs=3) as tpool,
                    tc.tile_pool(name="p5s", bufs=2) as spool,
                tc.tile_pool(name="p5w", bufs=1) as wpool,
                    tc.tile_pool(name="p5ps", bufs=2, space="PSUM") as pspool5,
                ):
                    st = {"ps2": None, "base": 0, "n": 0, "t0": 0, "t1": 0,
                          "h": [None, None], "c": [None, None],
                          "x1": [None, None]}

                    def lstm_layer_step(l, t, xT):
                        pg = pspool5.tile([H, 4 * BL], f32, space="PSUM",
                                          tag=f"g{l}", name=f"pg{l}")
                        nc.tensor.matmul(out=pg[:, 0:3 * BL], lhsT=bs3[l][:],
                                         rhs=gsel[:], start=True, stop=False,
                                         skip_group_check=True)
                        for q in range(3):
                            nc.tensor.matmul(
                                out=pg[:, q * BL:(q + 1) * BL],
                                lhsT=w_i[l][:, q * H:(q + 1) * H],
                                rhs=xT, start=False, stop=(t == 0),
                                skip_group_check=True)
                            if t > 0:
                                nc.tensor.matmul(
                                    out=pg[:, q * BL:(q + 1) * BL],
                                    lhsT=w_h[l][:, q * H:(q + 1) * H],
                                    rhs=st["h"][l][:], start=False, stop=True,
                                    skip_group_check=True)
                        nc.tensor.matmul(
                            out=pg[:, 3 * BL:4 * BL],
                            lhsT=w_i[l][:, 3 * H:4 * H],
                            rhs=xT, start=True, stop=(t == 0),
                            skip_group_check=True)
                        if t > 0:
                            nc.tensor.matmul(
                                out=pg[:, 3 * BL:4 * BL],
                                lhsT=w_h[l][:, 3 * H:4 * H],
                                rhs=st["h"][l][:], start=False, stop=True,
                                skip_group_check=True)
                        sig = wpool.tile([H, 3 * BL], bf, tag=f"sig{l}",
                                         name=f"sig{l}")
                        nc.scalar.activation(sig[:], pg[:, 0:3 * BL],
                                             AF.Sigmoid)
                        tg = wpool.tile([H, BL], bf, tag=f"tg{l}",
                                        name=f"tg{l}")
                        nc.scalar.activation(tg[:], pg[:, 3 * BL:4 * BL],
                                             AF.Tanh, bias=bg[l][:, 0:1])
                        t1 = wpool.tile([H, BL], bf, tag=f"t1{l}",
                                        name=f"t1{l}")
                        nc.vector.tensor_tensor(out=t1[:], in0=sig[:, 0:BL],
                                                in1=tg[:], op=ALU.mult)
                        cnew = spool.tile([H, BL], f32, tag=f"c{l}",
                                          name=f"c{l}")
                        if t > 0:
                            nc.vector.tensor_tensor(
                                out=cnew[:], in0=sig[:, BL:2 * BL],
                                in1=st["c"][l][:], op=ALU.mult)
                            nc.vector.tensor_tensor(
                                out=cnew[:], in0=cnew[:], in1=t1[:],
                                op=ALU.add)
                        else:
                            nc.vector.tensor_copy(out=cnew[:], in_=t1[:])
                        tc_ = wpool.tile([H, BL], bf, tag=f"tc{l}",
                                         name=f"tc{l}")
                        nc.scalar.activation(tc_[:], cnew[:], AF.Tanh)
                        hnew = spool.tile([H, BL], bf, tag=f"h{l}",
                                          name=f"h{l}")
                        nc.vector.tensor_tensor(out=hnew[:],
                                                in0=sig[:, 2 * BL:3 * BL],
                                                in1=tc_[:], op=ALU.mult)
                        st["c"][l] = cnew
                        st["h"][l] = hnew
                        if l == 1:
                            nc.vector.tensor_reduce(
                                out=pooledT[:, t:t + 1], in_=hnew[:],
                                axis=mybir.AxisListType.X, op=ALU.add)

                    def lstm_advance(valid_cols):
                        # l0 runs one step ahead of l1 so the two recurrence
                        # chains overlap across engines
                        while st["t0"] < T and (st["t0"] + 1) * BL <= valid_cols:
                            t = st["t0"]
                            lstm_layer_step(0, t, h2T[:, t * BL:(t + 1) * BL])
                            st["x1"][t % 2] = st["h"][0]
                            st["t0"] += 1
                            if st["t1"] < st["t0"] - 1:
                                t1_ = st["t1"]
                                lstm_layer_step(1, t1_, st["x1"][t1_ % 2][:])
                                st["t1"] += 1
                        if valid_cols >= SH:
                            while st["t1"] < st["t0"]:
                                t1_ = st["t1"]
                                lstm_layer_step(1, t1_, st["x1"][t1_ % 2][:])
                                st["t1"] += 1

                    def flush_h2(valid_cols):
                        if st["n"] == 0:
                            return
                        w = st["n"] * 128
                        nc.scalar.activation(
                            h2T[:, st["base"]:st["base"] + w],
                            st["ps2"][:, :w], AF.Relu, bias=b1c[:, 0:1])
                        st["base"] += w
                        st["n"] = 0
                        if _NO_LSTM:
                            return
                        lstm_advance(valid_cols)

                    def emit_group2(g, mms):
                        w = 128 if g < NG - 1 else SH - 128 * (NG - 1)
                        ps = pspool4a.tile([H, 128], f32, space="PSUM",
                                           tag="agg2")
                        for oh_ap, g_ap, first, last in mms:
                            nc.tensor.matmul(out=ps[:], lhsT=g_ap, rhs=oh_ap,
                                             start=first, stop=last)
                        aggS = tpool.tile([H, 128], bf, tag="aggS")
                        nc.vector.tensor_copy(out=aggS[:], in_=ps[:])
                        if st["n"] == 0:
                            st["ps2"] = pspool4b.tile([H, 512], f32, space="PSUM",
                                                      tag="h2", name="h2ps")
                        n = st["n"]
                        nc.tensor.matmul(out=st["ps2"][:, n * 128:n * 128 + 128],
                                         lhsT=w_s1[:],
                                         rhs=h1Tl[:, g * 128:g * 128 + 128],
                                         start=True, stop=False)
                        nc.tensor.matmul(out=st["ps2"][:, n * 128:n * 128 + 128],
                                         lhsT=w_n1[:], rhs=aggS[:],
                                         start=False, stop=True)
                        st["n"] += 1
                        if st["n"] == 4:
                            flush_h2(min(st["base"] + 512, SH))

                    def emit_sup2(sup_base, sup):
                        pass

                    conv_phase(gpool4, ohpool4, ipool4, h1tab,
                               emit_group2, emit_sup2, H)
                    flush_h2(SH)
                    if not _NO_LSTM:
                        lstm_advance(SH)

            if stop_after is None or stop_after >= 6:
                # ---------------- Phase 6: head ------------------------------
                with (
                    tc.tile_pool(name="p6", bufs=1) as hp,
                    tc.tile_pool(name="p6ps", bufs=1, space="PSUM") as psp,
                ):
                    psl = psp.tile([OUT, B_GRAPHS], f32, space="PSUM", tag="lg")
                    nc.tensor.matmul(out=psl[:], lhsT=w_o[:], rhs=pooledT[:],
                                     start=True, stop=True)
                    lgl = hp.tile([OUT, B_GRAPHS], f32)
                    nc.vector.tensor_copy(out=lgl[:], in_=psl[:])
                    nc.sync.dma_start(out=ccr_in[:], in_=lgl[:])
                    nc.gpsimd.collective_compute(
                        "AllReduce", mybir.AluOpType.add,
                        replica_groups=[list(range(NCORES))],
                        ins=[ccr_in.opt()], outs=[ccr_out.opt()],
                    )
                    lg = hp.tile([OUT, B_GRAPHS], f32)
                    nc.sync.dma_start(out=lg[:], in_=ccr_out[:])
                    nc.vector.tensor_scalar(out=lg[:], in0=lg[:],
                                            scalar1=b_o[:, 0:1], scalar2=None,
                                            op0=ALU.add)
                    pst = psp.tile([B_GRAPHS, OUT], f32, space="PSUM", tag="lgt")
                    nc.tensor.transpose(out=pst[:], in_=lg[:],
                                        identity=identf[0:OUT, 0:OUT])
                    z = hp.tile([B_GRAPHS, OUT], f32)
                    nc.vector.tensor_copy(out=z[:], in_=pst[:])
                    m = hp.tile([B_GRAPHS, 1], f32)
                    nc.vector.tensor_reduce(out=m[:], in_=z[:],
                                            axis=mybir.AxisListType.X, op=ALU.max)
                    negm = hp.tile([B_GRAPHS, 1], f32)
                    nc.vector.tensor_scalar(out=negm[:], in0=m[:], scalar1=-1.0,
                                            scalar2=None, op0=ALU.mult)
                    e = hp.tile([B_GRAPHS, OUT], f32)
                    se = hp.tile([B_GRAPHS, 1], f32)
                    nc.scalar.activation(e[:], z[:], AF.Exp, bias=negm[:, 0:1],
                                         accum_out=se[:])
                    ls = hp.tile([B_GRAPHS, 1], f32)
                    nc.scalar.activation(ls[:], se[:], AF.Ln)
                    o_sb = hp.tile([B_GRAPHS, OUT], f32)
                    nc.vector.tensor_scalar(out=o_sb[:], in0=z[:],
                                            scalar1=m[:, 0:1], scalar2=ls[:, 0:1],
                                            op0=ALU.subtract, op1=ALU.subtract)
                    nc.sync.dma_start(out=d_out[:], in_=o_sb[:])

    nc.compile()
    return nc


# --------------------------------------------------------------------------
# PJRT runner (built once, reused across calls)
# --------------------------------------------------------------------------
class _Runner:
    def __init__(self, nc, n_cores):
        import jax
        import concourse.mybir as mybir
        from jax.sharding import Mesh, PartitionSpec
        from jax.experimental.shard_map import shard_map
        from concourse.bass2jax import (
            _bass_exec_p, install_neuronx_cc_hook, partition_id_tensor)

        install_neuronx_cc_hook()
        self.n_cores = n_cores
        in_names, out_names, out_avals, zero_outs = [], [], [], []
        pname = nc.partition_id_tensor.name if nc.partition_id_tensor else None
        for alloc in nc.m.functions[0].allocations:
            if not isinstance(alloc, mybir.MemoryLocationSet):
                continue
            name = alloc.memorylocations[0].name
            if alloc.kind == "ExternalInput":
                if name != pname:
                    in_names.append(name)
            elif alloc.kind == "ExternalOutput":
                shape = tuple(alloc.tensor_shape)
                dtype = mybir.dt.np(alloc.dtype)
                out_names.append(name)
                out_avals.append(jax.core.ShapedArray(shape, dtype))
                zero_outs.append(np.zeros(shape, dtype))
        self.in_names, self.out_names = in_names, out_names
        self.out_avals, self.zero_outs = out_avals, zero_outs
        n_params, n_outs = len(in_names), len(out_names)
        all_in = list(in_names) + list(out_names) + ([pname] if pname else [])

        def _body(*args):
            operands = list(args)
            if pname is not None:
                operands.append(partition_id_tensor())
            return tuple(_bass_exec_p.bind(
                *operands, out_avals=tuple(out_avals),
                in_names=tuple(all_in), out_names=tuple(out_names),
                lowering_input_output_aliases=(),
                sim_require_finite=True, sim_require_nnan=True, nc=nc))

        devices = jax.devices()[:n_cores]
        mesh = Mesh(np.asarray(devices), ("core",))
        self._jax = jax
        self.sharded = jax.jit(
            shard_map(_body, mesh=mesh,
                      in_specs=(PartitionSpec("core"),) * (n_params + n_outs),
                      out_specs=(PartitionSpec("core"),) * n_outs,
                      check_rep=False),
            donate_argnums=tuple(range(n_params, n_params + n_outs)),
            keep_unused=True)

    def concat_inputs(self, in_maps):
        return [np.concatenate([np.asarray(m[nm]) for m in in_maps], axis=0)
                for nm in self.in_names]

    def run(self, concat_in):
        zeros = [np.zeros((self.n_cores * z.shape[0], *z.shape[1:]), z.dtype)
                 for z in self.zero_outs]
        out = self.sharded(*concat_in, *zeros)
        self._jax.block_until_ready(out)
        return out

    def split(self, out_arrs):
        return [{nm: np.asarray(out_arrs[i]).reshape(
            self.n_cores, *self.out_avals[i].shape)[c]
            for i, nm in enumerate(self.out_names)}
            for c in range(self.n_cores)]


def kernel(**inputs):
    shared, percore, meta, K, gmeta, NCH, chunk_base = _host_prep(inputs)
    if meta not in _BUILT:
        nc = _build_nc(K, gmeta, NCH, chunk_base)
        _BUILT[meta] = (nc, _Runner(nc, NCORES))
    nc, runner = _BUILT[meta]
    in_maps = [dict(shared, **percore[c]) for c in range(NCORES)]
    ci = runner.concat_inputs(in_maps)
    outs = runner.split(runner.run(ci))
    return np.asarray(outs[0]["out"], np.float32)

